# revision 1
# baseline (speedup 1.0000x reference)
"""BinaryConv2d on 8 TRN2 NeuronCores.

Problem: x (32,256,56,56) f32, weights (256,256,3,3) f32.
  out = conv2d(x, sign(weights)), NCHW/OIHW, stride 1, VALID -> (32,256,54,54).

Strategy (data-parallel): 4 images per core, weights (tiny, binarized)
replicated. On each core the conv is computed as 18 PSUM-accumulating
matmuls per output tile: 9 kernel taps x 2 input-channel tiles of 128.
  lhsT[c,o] = sign(W)[o,c,kh,kw]          (stationary, fp16, exact +-1)
  rhs[c, 9x54] = x[c, y0+kh : y0+kh+9, kw : kw+OW]  (moving, fp16)
  psum[o, 486] += lhsT.T @ rhs            (fp32 accumulation)
Free dim N = 9*54 = 486 <= 512 (one PSUM bank). 54 = 6 blocks of 9 rows.
fp16 (not bf16): binarized weights are exact either way, and fp16's 10
mantissa bits cut the x-rounding error ~8x at identical PE throughput.

Startup engineering: x input DMAs ride the sync-engine HWDGE queues and
weights + output DMAs ride the scalar-engine queues so they move in
parallel; x is split into row chunks and w into per-(ct,ot) quarters so
the first accumulation group's deps land early; a short dummy-matmul
warmup keeps the PE busy from the end of the framework preamble until
the first chunks land, so the HAM clock-gate is already at 8/8 when the
real stream starts. The final output block is split in two so its PSUM
drain + output DMA overlap the closing matmuls.
"""

import os
import sys

import numpy as np

for _p in ("/opt/trn_rl_repo", "/root/.axon_site/_ro/trn_rl_repo"):
    if os.path.isdir(_p) and _p not in sys.path:
        sys.path.insert(0, _p)

import concourse.bacc as bacc
import concourse.mybir as mybir
from concourse import tile
from concourse.bass_utils import run_bass_kernel_spmd

N_CORES = 8
B, C, H, W = 32, 256, 56, 56
O, KH, KW = 256, 3, 3
OH, OW = H - KH + 1, W - KW + 1  # 54, 54
BPC = B // N_CORES  # images per core
CT = C // 128  # input-channel tiles
OT = O // 128  # output-channel tiles
YR = 9  # output rows per matmul block
YB = OH // YR  # 6 blocks
NF = YR * OW  # 486 free dim
NKK = KH * KW  # 9 taps
# x row chunks: yb block j reads input rows [9j, 9j+11). Chunk boundaries
# chosen so the first matmuls' data lands as early as possible.
XCHUNKS = (0, 11, 20, 29, 56)
WARMUP_MM = 8  # dummy matmuls to lift the PE HAM clock-gate during load;
# sized to keep the PE continuously busy from the end of the framework
# preamble (~7.8us) until the first input chunks land (~10.8us), so the
# HAM activity window never sees an idle gap before the real stream.
# (Early DMA delivery runs at only ~150GB/s aggregate while the DGE
# descriptor path ramps, so the first chunks cannot usefully land sooner;
# finer-grained first chunks were measured to only move the stall.)

_NC_CACHE = {}


def _build():
    nc = bacc.Bacc("TRN2", target_bir_lowering=False, debug=False)
    fp16 = mybir.dt.float16
    x_d = nc.dram_tensor("x", [BPC, C, H, W], fp16, kind="ExternalInput")
    w_d = nc.dram_tensor("w", [CT, OT, 128, NKK, 128], fp16, kind="ExternalInput")
    out_d = nc.dram_tensor(
        "out", [BPC, O, OH, OW], mybir.dt.float32, kind="ExternalOutput"
    )
    x_ap = x_d.ap()
    w_ap = w_d.ap()
    out_flat = out_d.ap().rearrange("b o h w -> b o (h w)")

    with tile.TileContext(nc) as tc:
        with (
            tc.tile_pool(name="wpool", bufs=1) as wpool,
            tc.tile_pool(name="xpool", bufs=2) as xpool,
            tc.tile_pool(name="opool", bufs=4) as opool,
            tc.tile_pool(name="pspool", bufs=6, space="PSUM") as pspool,
            tc.tile_pool(name="pswarm", bufs=1, space="PSUM") as pswarm,
        ):
            # PE warmup: HAM un-throttles after ~3.4us of sustained PE work.
            # Burn dummy matmuls on a zero tile while the input DMAs land so
            # the real matmul stream starts at 2.4 GHz instead of 1.2.
            # (A dependency-free warmup on an uninitialized tile would start
            # ~1.4us earlier still, but the simulator rejects the read.)
            zt = wpool.tile([128, 512], fp16, tag="warm")
            nc.gpsimd.memset(zt[:], 0.0)
            wps = pswarm.tile([128, 512], mybir.dt.float32)
            for _ in range(WARMUP_MM):
                nc.tensor.matmul(wps[:], zt[:, :128], zt[:], start=True, stop=True)

            def x_load(n):
                """Load image n (n>=1): the plain tile plus a one-column-
                shifted copy. SBUF matmul reads are 4-byte granular, so the
                kw=1 tap's 2-byte (one fp16 column) offset costs +8ns per
                matmul; kw=1 reads the shifted copy at an aligned offset.
                The copy rides the mostly-idle Vector engine with ~45us of
                prefetch slack."""
                xts, xos = [], []
                for ct in range(CT):
                    xt = xpool.tile([128, H, W], fp16, tag=f"x{ct}")
                    xts.append(xt)
                for lo, hi in zip(XCHUNKS, XCHUNKS[1:]):  # top chunks first
                    for ct in range(CT):
                        nc.sync.dma_start(
                            xts[ct][:, lo:hi], x_ap[n, ct * 128 : (ct + 1) * 128, lo:hi]
                        )
                for ct in range(CT):
                    xo = xpool.tile([128, H, W], fp16, tag=f"xo{ct}")
                    nc.vector.tensor_copy(xo[:, :, 0 : W - 1], xts[ct][:, :, 1:W])
                    xos.append(xo)
                return xts, xos

            # x rides the sync-engine HWDGE queues, weights + outputs ride
            # the scalar-engine queues, so input streams move in parallel
            # (they share the core's HBM bandwidth either way). Image 0's
            # chunks are issued ct0-first to match the ct0-first matmul
            # order below; deadlines checked against the ~165GB/s early
            # aggregate DMA rate.
            x0ts = [
                xpool.tile([128, H, W], fp16, tag="x0", name="x0t_first"),
                xpool.tile([128, H, W], fp16, tag="x1", name="x1t_first"),
            ]

            def x0_chunk(ct, ci):
                lo, hi = XCHUNKS[ci], XCHUNKS[ci + 1]
                nc.sync.dma_start(
                    x0ts[ct][:, lo:hi], x_ap[0, ct * 128 : (ct + 1) * 128, lo:hi]
                )

            for ct, ci in ((0, 0), (0, 1), (0, 2), (1, 0), (0, 3), (1, 1), (1, 2), (1, 3)):
                x0_chunk(ct, ci)
            w_sb = wpool.tile([128, CT, OT, NKK, 128], fp16)
            for ot in range(OT):  # first group is ot=0: load its halves first
                for ct in range(CT):
                    nc.scalar.dma_start(w_sb[:, ct, ot], w_ap[ct, ot])

            def emit_group(xts, n, ot, y0, rows, xos=None):
                ps = pspool.tile([128, rows * OW], mybir.dt.float32, tag="ps")
                k = 0
                for ct in range(CT):
                    for kh in range(KH):
                        for kw in range(KW):
                            if kw == 1 and xos is not None:
                                rhs = xos[ct][:, y0 + kh : y0 + kh + rows, 0:OW]
                            else:
                                rhs = xts[ct][:, y0 + kh : y0 + kh + rows, kw : kw + OW]
                            nc.tensor.matmul(
                                ps[:],
                                w_sb[:, ct, ot, kh * KW + kw, :],
                                rhs,
                                start=(k == 0),
                                stop=(k == KH * KW * CT - 1),
                            )
                            k += 1
                ob = opool.tile([128, rows * OW], mybir.dt.float32, tag="ob")
                nc.vector.tensor_copy(ob[:], ps[:])
                nc.scalar.dma_start(
                    out_flat[
                        n, ot * 128 : (ot + 1) * 128, y0 * OW : (y0 + rows) * OW
                    ],
                    ob[:],
                )

            # First three blocks of image 0: run all ct=0 taps of all three
            # before any ct=1 tap (interleaved PSUM accumulation groups on
            # three banks). The ct=0 chunks land first on the ramping DMA
            # queues; this pushes the ct=1 dependency deadline ~5.5us later,
            # making the startup schedule feasible at the early DMA rate and
            # removing the measured stall at matmul #9.
            pre = [
                pspool.tile([128, NF], mybir.dt.float32, tag="ps", name=f"ps_pre{i}")
                for i in range(3)
            ]
            for ct in range(CT):
                for yb in range(3):
                    y0 = yb * YR
                    for kh in range(KH):
                        for kw in range(KW):
                            nc.tensor.matmul(
                                pre[yb][:],
                                w_sb[:, ct, 0, kh * KW + kw, :],
                                x0ts[ct][:, y0 + kh : y0 + kh + YR, kw : kw + OW],
                                start=(ct == 0 and kh == 0 and kw == 0),
                                stop=(ct == CT - 1 and kh == KH - 1 and kw == KW - 1),
                            )
            for yb in range(3):
                ob = opool.tile(
                    [128, NF], mybir.dt.float32, tag="ob", name=f"ob_pre{yb}"
                )
                nc.vector.tensor_copy(ob[:], pre[yb][:])
                nc.scalar.dma_start(
                    out_flat[0, 0:128, yb * YR * OW : (yb + 1) * YR * OW], ob[:]
                )

            for n in range(BPC):
                if n == 0:
                    xts, xos = x0ts, None  # startup-critical: unaligned kw=1
                else:
                    xts, xos = x_load(n)
                for ot in range(OT):
                    for yb in range(YB):
                        if n == 0 and ot == 0 and yb < 3:
                            continue  # emitted above
                        last = n == BPC - 1 and ot == OT - 1 and yb == YB - 1
                        if not last:
                            emit_group(xts, n, ot, yb * YR, YR, xos)
                        else:
                            # Split the final block by rows so its PSUM drain +
                            # output DMA overlap the closing matmuls.
                            emit_group(xts, n, ot, yb * YR, 5, xos)
                            emit_group(xts, n, ot, yb * YR + 5, 4, xos)
    nc.compile()
    return nc


def get_nc():
    if "nc" not in _NC_CACHE:
        _NC_CACHE["nc"] = _build()
    return _NC_CACHE["nc"]


def prep_inputs(x, weights):
    """Full f32 inputs -> per-core in_maps (fp16)."""
    x = np.ascontiguousarray(np.asarray(x, dtype=np.float32))
    weights = np.asarray(weights, dtype=np.float32)
    qw = np.sign(weights).astype(np.float32)  # [O, I, KH, KW]
    w6 = qw.reshape(OT, 128, CT, 128, KH, KW)  # [ot, o, ct, c, kh, kw]
    wt = np.transpose(w6, (2, 0, 3, 4, 5, 1))  # [ct, ot, c, kh, kw, o]
    w5 = np.ascontiguousarray(wt).reshape(CT, OT, 128, NKK, 128).astype(np.float16)
    x_f16 = x.reshape(N_CORES, BPC, C, H, W).astype(np.float16)
    return [{"x": x_f16[i], "w": w5} for i in range(N_CORES)]


def run_spmd(in_maps, **kwargs):
    nc = get_nc()
    return run_bass_kernel_spmd(nc, in_maps, list(range(N_CORES)), **kwargs)


def kernel(x, weights):
    in_maps = prep_inputs(x, weights)
    res = run_spmd(in_maps)
    out = np.concatenate(
        [np.asarray(res.results[i]["out"]) for i in range(N_CORES)], axis=0
    )
    return np.ascontiguousarray(out.astype(np.float32))



# revision 3
# speedup vs baseline: 1.2495x; 1.2495x over previous
"""BinaryConv2d on 8 TRN2 NeuronCores via 1D Winograd F(2,3) along W.

Problem: x (32,256,56,56) f32, weights (256,256,3,3) f32.
  out = conv2d(x, sign(weights)), NCHW/OIHW, stride 1, VALID -> (32,256,54,54).

Strategy (data-parallel): 4 images per core, weights replicated. The W
dimension is Winograd-transformed with F(2,3): for each output column
pair (2j, 2j+1) the 3-tap row conv becomes 4 multiplies instead of 6,
cutting PE work 1.5x vs direct conv (175us -> 116.6us fp16 floor).
  T[c,y,j,t]: t0=d0-d2, t1=d1+d2, t2=d2-d1, t3=d1-d3   (dk = x[c,y,2j+k])
  ghat[o,c,kh,t] = [g0, (g0+g1+g2)/2, (g0-g1+g2)/2, g2] (g = sign(w)[o,c,kh,:])
  m[o,y,j,t] = sum_{c,kh} ghat[o,c,kh,t] * T[c,y+kh,j,t]  (PE, fp32 PSUM)
  out[o,y,2j]   = m0+m1+m2;  out[o,y,2j+1] = m1-m2-m3     (DVE)
ghat is exact in fp16 (+-1, +-1/2, +-3/2) because the weights are binary;
T costs one fp16 rounding on an add of two x values, so accuracy stays at
the fp16 level (measured 3.8e-4 rel err vs the 2e-2 gate).

Per (img, ot, 18-row block): 24 PSUM-accumulating matmuls (4 taps x 2
input-channel tiles x 3 kh) of free dim 18*27=486 into 4 PSUM banks
(t0..t3), double-buffered = all 8 banks. Engine split: sync-DGE queues
carry x, scalar-DGE queues carry weights + outputs, GpSimd does the
input transform (4 tensor_tensor ops per x row-chunk, row-local so it
chases the DMA), DVE does the output transform (4 ops per group writing
even/odd columns through stride-2 views).

Startup engineering (carried over from the direct-conv baseline): x is
split into row chunks so the first accumulation groups' deps land early;
a short dummy-matmul warmup keeps the PE busy from the end of the
framework preamble until the first chunks land; the first two groups run
all ct=0 taps before any ct=1 tap to push the ct=1 DMA deadline out; the
final block is split in two so its PSUM drain + output DMA overlap the
closing matmuls.
"""

import os
import sys

import numpy as np

for _p in ("/opt/trn_rl_repo", "/root/.axon_site/_ro/trn_rl_repo"):
    if os.path.isdir(_p) and _p not in sys.path:
        sys.path.insert(0, _p)

import concourse.bacc as bacc
import concourse.mybir as mybir
from concourse import tile
from concourse.bass_utils import run_bass_kernel_spmd

N_CORES = 8
B, C, H, W = 32, 256, 56, 56
O, KH, KW = 256, 3, 3
OH, OW = H - KH + 1, W - KW + 1  # 54, 54
BPC = B // N_CORES  # images per core
CT = C // 128  # input-channel tiles
OT = O // 128  # output-channel tiles
NT = 4  # Winograd taps along W for F(2,3)
J = OW // 2  # 27 output column pairs
JP = J + 1  # T inner dim padded to 28 so row stride is 4-byte aligned
YR = 18  # output rows per matmul block
YB = OH // YR  # 3 blocks
NF = YR * J  # 486 free dim
# x row chunks: block yb reads input rows [18*yb, 18*yb+20).
XCHUNKS = (0, 11, 20, 29, 56)
WARMUP_MM = 8  # dummy matmuls to lift the PE HAM clock-gate during load

_ADD = mybir.AluOpType.add
_SUB = mybir.AluOpType.subtract

_NC_CACHE = {}


def _build():
    nc = bacc.Bacc("TRN2", target_bir_lowering=False, debug=False)
    fp16 = mybir.dt.float16
    f32 = mybir.dt.float32
    x_d = nc.dram_tensor("x", [BPC, C, H, W], fp16, kind="ExternalInput")
    w_d = nc.dram_tensor("w", [CT, OT, 128, NT, KH, 128], fp16, kind="ExternalInput")
    out_d = nc.dram_tensor("out", [BPC, O, OH, OW], f32, kind="ExternalOutput")
    x_ap = x_d.ap()
    w_ap = w_d.ap()
    out_flat = out_d.ap().rearrange("b o h w -> b o (h w)")

    with tile.TileContext(nc) as tc:
        with (
            tc.tile_pool(name="wpool", bufs=1) as wpool,
            tc.tile_pool(name="xpool", bufs=2) as xpool,
            tc.tile_pool(name="opool", bufs=4) as opool,
            tc.tile_pool(name="pspool", bufs=2, space="PSUM") as pspool,
        ):
            # PE warmup: HAM un-throttles after ~3.4us of sustained PE work.
            # Burn dummy matmuls on a zero tile while the input DMAs land so
            # the real matmul stream starts at 2.4 GHz instead of 1.2.
            zt = wpool.tile([128, 512], fp16, tag="warm")
            nc.gpsimd.memset(zt[:], 0.0)
            wps = pspool.tile([128, 512], f32, tag="t0", name="wps")
            for _ in range(WARMUP_MM):
                nc.tensor.matmul(wps[:], zt[:, :128], zt[:], start=True, stop=True)

            def x_issue_dma(n, xts, order):
                for ct, ci in order:
                    lo, hi = XCHUNKS[ci], XCHUNKS[ci + 1]
                    nc.sync.dma_start(
                        xts[ct][:, lo:hi], x_ap[n, ct * 128 : (ct + 1) * 128, lo:hi]
                    )

            def x_transform(n, xts, order):
                """F(2,3) input transform on GpSimd, chunk by chunk as the
                x DMAs land (row-local: T row y only needs x row y)."""
                Ts = [
                    xpool.tile([128, NT, H, JP], fp16, tag=f"T{ct}", name=f"T{ct}_{n}")
                    for ct in range(CT)
                ]
                for ct, ci in order:
                    lo, hi = XCHUNKS[ci], XCHUNKS[ci + 1]
                    xv = xts[ct].rearrange("p h (j two) -> p h j two", two=2)
                    d0 = xv[:, lo:hi, 0:J, 0]
                    d1 = xv[:, lo:hi, 0:J, 1]
                    d2 = xv[:, lo:hi, 1 : J + 1, 0]
                    d3 = xv[:, lo:hi, 1 : J + 1, 1]
                    Tt = Ts[ct]
                    nc.gpsimd.tensor_tensor(Tt[:, 0, lo:hi, 0:J], d0, d2, _SUB)
                    nc.gpsimd.tensor_tensor(Tt[:, 1, lo:hi, 0:J], d1, d2, _ADD)
                    nc.gpsimd.tensor_tensor(Tt[:, 2, lo:hi, 0:J], d2, d1, _SUB)
                    nc.gpsimd.tensor_tensor(Tt[:, 3, lo:hi, 0:J], d1, d3, _SUB)
                return Ts

            # Image 0's chunks are issued ct0-first to match the ct0-first
            # matmul order below.
            x0ts = [
                xpool.tile([128, H, W], fp16, tag=f"x{ct}", name=f"x{ct}t_first")
                for ct in range(CT)
            ]
            order0 = ((0, 0), (0, 1), (1, 0), (0, 2), (1, 1), (0, 3), (1, 2), (1, 3))
            x_issue_dma(0, x0ts, order0)
            T0s = x_transform(0, x0ts, order0)

            # Weights ride the scalar-engine queues; (ct0, ot0) taps first in
            # matmul order.
            w_sb = wpool.tile([128, CT, OT, NT, KH, 128], fp16)
            for ot in range(OT):
                for ct in range(CT):
                    for t in range(NT):
                        nc.scalar.dma_start(w_sb[:, ct, ot, t], w_ap[ct, ot, :, t])

            def drain_group(ps, n, ot, y0, rows, name):
                """A^T m on DVE: even cols = m0+m1+m2, odd = m1-m2-m3.
                Ops read at most one PSUM operand each (hw restriction), so
                m1 is staged through SBUF."""
                ob = opool.tile([128, rows, OW], f32, tag="ob", name=f"ob_{name}")
                a = opool.tile([128, rows, J], f32, tag="ta", name=f"ta_{name}")
                b = opool.tile([128, rows, J], f32, tag="tb", name=f"tb_{name}")
                c = opool.tile([128, rows, J], f32, tag="tc", name=f"tc_{name}")
                obr = ob.rearrange("p r (j two) -> p r j two", two=2)
                nc.vector.tensor_copy(a[:], ps[1][:])
                nc.vector.tensor_tensor(b[:], a[:], ps[2][:], _ADD)
                nc.vector.tensor_tensor(obr[:, :, :, 0], b[:], ps[0][:], _ADD)
                nc.vector.tensor_tensor(c[:], a[:], ps[2][:], _SUB)
                nc.vector.tensor_tensor(obr[:, :, :, 1], c[:], ps[3][:], _SUB)
                nc.scalar.dma_start(
                    out_flat[n, ot * 128 : (ot + 1) * 128, y0 * OW : (y0 + rows) * OW],
                    ob[:],
                )

            def emit_group(Ts, n, ot, y0, rows, name):
                ps = [
                    pspool.tile([128, rows, J], f32, tag=f"t{t}", name=f"ps{t}_{name}")
                    for t in range(NT)
                ]
                for ct in range(CT):
                    for kh in range(KH):
                        for t in range(NT):
                            nc.tensor.matmul(
                                ps[t][:],
                                w_sb[:, ct, ot, t, kh, :],
                                Ts[ct][:, t, y0 + kh : y0 + kh + rows, 0:J],
                                start=(ct == 0 and kh == 0),
                                stop=(ct == CT - 1 and kh == KH - 1),
                            )
                drain_group(ps, n, ot, y0, rows, name)

            # First two blocks of image 0: run all ct=0 taps of both before
            # any ct=1 tap (interleaved accumulation on all 8 PSUM banks), so
            # the ct=1 chunks' DMA deadline moves ~5us later on the ramping
            # DMA queues.
            pre = [
                [
                    pspool.tile([128, YR, J], f32, tag=f"t{t}", name=f"ps{t}_pre{g}")
                    for t in range(NT)
                ]
                for g in range(2)
            ]
            for ct in range(CT):
                for g in range(2):
                    y0 = g * YR
                    for kh in range(KH):
                        for t in range(NT):
                            nc.tensor.matmul(
                                pre[g][t][:],
                                w_sb[:, ct, 0, t, kh, :],
                                T0s[ct][:, t, y0 + kh : y0 + kh + YR, 0:J],
                                start=(ct == 0 and kh == 0),
                                stop=(ct == CT - 1 and kh == KH - 1),
                            )
            for g in range(2):
                drain_group(pre[g], 0, 0, g * YR, YR, f"pre{g}")

            for n in range(BPC):
                if n == 0:
                    Ts = T0s
                else:
                    xts = [
                        xpool.tile([128, H, W], fp16, tag=f"x{ct}", name=f"x{ct}t_{n}")
                        for ct in range(CT)
                    ]
                    order = tuple((ct, ci) for ci in range(4) for ct in range(CT))
                    x_issue_dma(n, xts, order)
                    Ts = x_transform(n, xts, order)
                for ot in range(OT):
                    for yb in range(YB):
                        if n == 0 and ot == 0 and yb < 2:
                            continue  # emitted above
                        last = n == BPC - 1 and ot == OT - 1 and yb == YB - 1
                        if not last:
                            emit_group(Ts, n, ot, yb * YR, YR, f"{n}_{ot}_{yb}")
                        else:
                            # Split the final block so its PSUM drain + output
                            # DMA overlap the closing matmuls.
                            emit_group(Ts, n, ot, yb * YR, 9, f"{n}_{ot}_{yb}a")
                            emit_group(Ts, n, ot, yb * YR + 9, 9, f"{n}_{ot}_{yb}b")
    nc.compile()
    return nc


def get_nc():
    if "nc" not in _NC_CACHE:
        _NC_CACHE["nc"] = _build()
    return _NC_CACHE["nc"]


def prep_inputs(x, weights):
    """Full f32 inputs -> per-core in_maps (fp16, Winograd weights)."""
    x = np.ascontiguousarray(np.asarray(x, dtype=np.float32))
    weights = np.asarray(weights, dtype=np.float32)
    qw = np.sign(weights).astype(np.float32)  # [O, C, KH, KW]
    g0, g1, g2 = qw[..., 0], qw[..., 1], qw[..., 2]
    gh = np.stack(
        [g0, (g0 + g1 + g2) / 2, (g0 - g1 + g2) / 2, g2], axis=-1
    )  # [O, C, KH, NT], exact in fp16
    gh6 = gh.reshape(OT, 128, CT, 128, KH, NT)  # [ot, o, ct, c, kh, t]
    wt = np.transpose(gh6, (2, 0, 3, 5, 4, 1))  # [ct, ot, c, t, kh, o]
    w6 = np.ascontiguousarray(wt).astype(np.float16)
    x_f16 = x.reshape(N_CORES, BPC, C, H, W).astype(np.float16)
    return [{"x": x_f16[i], "w": w6} for i in range(N_CORES)]


def run_spmd(in_maps, **kwargs):
    nc = get_nc()
    return run_bass_kernel_spmd(nc, in_maps, list(range(N_CORES)), **kwargs)


def kernel(x, weights):
    in_maps = prep_inputs(x, weights)
    res = run_spmd(in_maps)
    out = np.concatenate(
        [np.asarray(res.results[i]["out"]) for i in range(N_CORES)], axis=0
    )
    return np.ascontiguousarray(out.astype(np.float32))
